# revision 8
# baseline (speedup 1.0000x reference)
"""Trainium2 Bass kernel for nn_AttentionCircuit (mixture-routed attention).

Sharding (8 cores, SPMD single program):
  - mixing (project+combine) token-sharded: core c -> batch c//4, tokens [(c%4)*512, +512)
  - tiny AllGather of h vectors (h_q/h_k/h_v, [64,512] each) within 4-core batch groups
  - restore + attention + W_O head-sharded: core handles 4 heads (via per-core
    sliced R_qk/R_v/W_O inputs) over all 2048 tokens of its batch
  - host sums the 4 partial W_O outputs per batch
"""
import sys
sys.path.insert(0, "/opt/trn_rl_repo")
import numpy as np
from contextlib import ExitStack

import concourse.bacc as bacc
import concourse.mybir as mybir
from concourse import tile
from concourse.masks import make_identity
from concourse.bass_utils import run_bass_kernel_spmd

B, S, D, R, H, DH, N = 2, 2048, 1024, 64, 16, 64, 32
NR = N * R            # 2048
P = 128
TOK = 512             # tokens per core (mixing shard)
HL = 4                # local heads per core
DL = HL * DH          # 256 local head dims
GROUPS = [[0, 1, 2, 3], [4, 5, 6, 7]]
F32 = mybir.dt.float32
F32R = mybir.dt.float32r
MULT = mybir.AluOpType.mult
ADD = mybir.AluOpType.add
AXX = mybir.AxisListType.X
EXP = mybir.ActivationFunctionType.Exp

_CACHED = {}


def _r(ap):
    return ap.bitcast(F32R)


def build():
    nc = bacc.Bacc(None, target_bir_lowering=False)
    dp = lambda name, shape, out=False: nc.declare_dram_parameter(
        name, list(shape), F32, isOutput=out)

    xT_d = dp("xT", [D, TOK])
    fw_d = [dp(n, [TOK, N]) for n in ("fwq", "fwk", "fwv")]
    rwT_d = [dp(n, [N, S]) for n in ("rwqT", "rwkT", "rwvT")]
    Fqk_d = dp("Fqk", [D, NR])
    Fv_d = dp("Fv", [D, NR])
    Rqk_d = dp("Rqk", [NR, DL])
    Rv_d = dp("Rv", [NR, DL])
    WOT_d = dp("WOTs", [DL, D])
    maskU_d = dp("maskU", [P, P])
    out_d = dp("outp", [S, D], out=True)

    tog = [0]

    def cp(out, in_):
        tog[0] ^= 1
        if tog[0]:
            nc.scalar.copy(out, in_)
        else:
            nc.vector.tensor_copy(out, in_)

    with ExitStack() as ctx:
        tc = ctx.enter_context(tile.TileContext(nc))
        const = ctx.enter_context(tc.tile_pool(name="const", bufs=1))
        ident = const.tile([P, P], F32, name="ident")
        make_identity(nc, ident[:])
        maskU = const.tile([P, P], F32, name="maskU")
        nc.sync.dma_start(out=maskU[:], in_=maskU_d[:])

        dram = ctx.enter_context(tc.tile_pool(name="dram", bufs=1, space="DRAM"))
        hT_stack = dram.tile([3 * 64, TOK], F32, name="hTstack")
        hT_gath = dram.tile([4 * 3 * 64, TOK], F32, name="hTgath")

        hpool = ctx.enter_context(tc.tile_pool(name="hpool", bufs=12))
        hTpool = ctx.enter_context(tc.tile_pool(name="hTpool", bufs=3))
        h_sb = {}     # (tensor, u) -> [P, R]

        # ---------------- Phase A/B: projections + combines ----------------
        with tc.tile_pool(name="xF", bufs=8) as xF, \
             tc.tile_pool(name="fw", bufs=12) as fwp, \
             tc.tile_pool(name="tmp", bufs=4) as tmpp, \
             tc.tile_pool(name="psA", bufs=4, space="PSUM") as psA, \
             tc.tile_pool(name="psH", bufs=2, space="PSUM") as psH:
            xT_sb = []
            for d in range(8):
                t = xF.tile([P, TOK], F32, tag="xT", name="xT")
                nc.sync.dma_start(out=_r(t[:]), in_=_r(xT_d[d * P:(d + 1) * P, :]))
                xT_sb.append(t)
            fw_sb = {}
            for ti in range(3):
                for u in range(4):
                    t = fwp.tile([P, N], F32, tag="fw", name="fw")
                    nc.sync.dma_start(out=t[:], in_=fw_d[ti][u * P:(u + 1) * P, :])
                    fw_sb[(ti, u)] = t

            for phase, (F_d, tensors) in enumerate(
                    [(Fqk_d, (0, 1)), (Fv_d, (2,))]):
                F_sb = {}
                for d in range(8):
                    for ns in range(4):
                        t = xF.tile([P, 512], F32, tag="F", name="F", bufs=32)
                        nc.sync.dma_start(
                            out=_r(t[:]),
                            in_=_r(F_d[d * P:(d + 1) * P, ns * 512:(ns + 1) * 512]))
                        F_sb[(d, ns)] = t
                for u in range(4):
                    tmps = {ti: tmpp.tile([P, NR], F32, tag="tmp", name="tmp") for ti in tensors}
                    for ns in range(4):
                        ps = psA.tile([P, 512], F32, name="psA")
                        for d in range(8):
                            nc.tensor.matmul(
                                ps[:], _r(xT_sb[d][:, u * P:(u + 1) * P]),
                                _r(F_sb[(d, ns)][:]),
                                start=(d == 0), stop=(d == 7))
                        p3 = ps[:].rearrange("p (n r) -> p n r", n=8)
                        for ti in tensors:
                            w3 = fw_sb[(ti, u)][:, ns * 8:(ns + 1) * 8] \
                                .unsqueeze(2).broadcast_to([P, 8, R])
                            tv = tmps[ti][:].rearrange("p (r n) -> p n r", r=R)[
                                :, ns * 8:(ns + 1) * 8, :]
                            nc.vector.tensor_tensor(out=tv, in0=p3, in1=w3, op=MULT)
                    for ti in tensors:
                        h = hpool.tile([P, R], F32, tag="h", name="h")
                        nc.vector.reduce_sum(
                            out=h[:],
                            in_=tmps[ti][:].rearrange("p (r n) -> p r n", r=R),
                            axis=AXX)
                        h_sb[(ti, u)] = h

            # transpose h -> hT [64, TOK] and stage for AllGather
            for ti in range(3):
                hT = hTpool.tile([64, TOK], F32, tag="hT", name="hT")
                for u in range(4):
                    tp = psH.tile([R, P], F32, name="psH")
                    nc.tensor.transpose(tp[:], h_sb[(ti, u)][:], ident[:])
                    cp(hT[:, u * P:(u + 1) * P], tp[:])
                nc.sync.dma_start(out=hT_stack[ti * 64:(ti + 1) * 64, :], in_=hT[:])

        nc.gpsimd.collective_compute(
            "AllGather", mybir.AluOpType.bypass, replica_groups=GROUPS,
            ins=[hT_stack.opt()], outs=[hT_gath.opt()])

        # h2[tensor] [P, S]: rows 0-63 and 64-127 both = gathered hT rows
        h2pool = ctx.enter_context(tc.tile_pool(name="h2", bufs=3))
        h2 = []
        gv = hT_gath[:].rearrange("(q kr) t -> q kr t", q=4)
        for ti in range(3):
            t = h2pool.tile([P, S], F32, name="h2")
            src = gv[:, ti * 64:(ti + 1) * 64, :].rearrange("q r t -> r q t")
            for half in range(2):
                nc.sync.dma_start(
                    out=t[half * 64:(half + 1) * 64, :]
                        .rearrange("p (q t) -> p q t", q=4),
                    in_=src)
            h2.append(t)

        # ---------------- Phase C/D: restores (local heads only) ----------------
        qkv_pool = ctx.enter_context(tc.tile_pool(name="qkv", bufs=2))
        QT_sb = [qkv_pool.tile([P, S], F32, tag="QT", name="QT", bufs=2) for _ in range(2)]
        KT_sb = [qkv_pool.tile([P, S], F32, tag="KT", name="KT", bufs=2) for _ in range(2)]
        V_sb = [qkv_pool.tile([P, DL], F32, tag="V", name="V", bufs=16) for _ in range(16)]

        with tc.tile_pool(name="Rp", bufs=16) as Rp, \
             tc.tile_pool(name="gT", bufs=18) as gTp, \
             tc.tile_pool(name="wrep", bufs=4) as wrp, \
             tc.tile_pool(name="psC", bufs=4, space="PSUM") as psC:
            R_sb = {}
            for k in range(16):
                t = Rp.tile([P, DL], F32, tag="R", name="R")
                nc.sync.dma_start(out=_r(t[:]),
                                  in_=_r(Rqk_d[k * P:(k + 1) * P, :]))
                R_sb[k] = t

            def grow_gT(ti, ch):
                tiles = []
                for k in range(16):
                    wr = wrp.tile([P, 512], F32, tag="wr", name="wr")
                    for half in range(2):
                        nn = 2 * k + half
                        nc.sync.dma_start(
                            out=wr[half * 64:(half + 1) * 64, :],
                            in_=rwT_d[ti][nn:nn + 1, ch * 512:(ch + 1) * 512]
                                .broadcast_to([64, 512]))
                    g = gTp.tile([P, 512], F32, tag="gT", name="gT")
                    nc.vector.tensor_mul(_r(g[:]), h2[ti][:, ch * 512:(ch + 1) * 512],
                                         wr[:])
                    tiles.append(g)
                return tiles

            for ti, outs in ((0, QT_sb), (1, KT_sb)):
                for ch in range(4):
                    gT = grow_gT(ti, ch)
                    for dt2 in range(2):
                        ps = psC.tile([P, 512], F32, name="psC")
                        for k in range(16):
                            nc.tensor.matmul(
                                ps[:], _r(R_sb[k][:, dt2 * P:(dt2 + 1) * P]),
                                _r(gT[k][:]), start=(k == 0), stop=(k == 15))
                        cp(
                            _r(outs[dt2][:, ch * 512:(ch + 1) * 512]), ps[:])
            # V (token-major), reload Rv into same slots
            for k in range(16):
                t = Rp.tile([P, DL], F32, tag="R", name="R")
                nc.sync.dma_start(out=_r(t[:]), in_=_r(Rv_d[k * P:(k + 1) * P, :]))
                R_sb[k] = t
            for ch in range(4):
                gT = grow_gT(2, ch)
                for tt in range(4):
                    ps = psC.tile([P, DL], F32, name="psCv")
                    for k in range(16):
                        nc.tensor.matmul(
                            ps[:], _r(gT[k][:, tt * P:(tt + 1) * P]),
                            _r(R_sb[k][:]), start=(k == 0), stop=(k == 15))
                    cp(_r(V_sb[ch * 4 + tt][:]), ps[:])

        # ---------------- Phase E: attention + W_O ----------------
        wot_pool = ctx.enter_context(tc.tile_pool(name="wot", bufs=2))
        WOT_sb = []
        for pr in range(2):
            t = wot_pool.tile([P, D], F32, name="wot")
            nc.sync.dma_start(out=_r(t[:]), in_=_r(WOT_d[pr * P:(pr + 1) * P, :]))
            WOT_sb.append(t)

        with tc.tile_pool(name="Ssb", bufs=2) as Sp, \
             tc.tile_pool(name="expS", bufs=2) as Ep, \
             tc.tile_pool(name="expT", bufs=4) as Tp, \
             tc.tile_pool(name="attnP", bufs=4) as Ap, \
             tc.tile_pool(name="osb", bufs=4) as Op, \
             tc.tile_pool(name="small", bufs=24) as smp, \
             tc.tile_pool(name="psS", bufs=2, space="PSUM") as psS, \
             tc.tile_pool(name="psT", bufs=2, space="PSUM") as psT, \
             tc.tile_pool(name="psAV", bufs=2, space="PSUM") as psAV, \
             tc.tile_pool(name="psWO", bufs=2, space="PSUM") as psWO:
            for qt in range(16):
                L = (qt + 1) * P
                nb = (L + 511) // 512
                pair = [Ap.tile([P, P], F32, tag="ap", name="ap") for _ in range(2)]
                for i in range(HL):
                    qtile, qoff = QT_sb[i // 2], (i % 2) * 64
                    ktile = KT_sb[i // 2]
                    S_sb = Sp.tile([P, S], F32, tag="S", name="S")
                    mxs = []
                    for kb in range(nb):
                        Ls = min(512, L - kb * 512)
                        ps = psS.tile([P, 512], F32, name="psS")
                        nc.tensor.matmul(
                            ps[:, :Ls],
                            _r(qtile[qoff:qoff + 64, qt * P:(qt + 1) * P]),
                            _r(ktile[qoff:qoff + 64, kb * 512:kb * 512 + Ls]),
                            start=True, stop=True)
                        nc.vector.scalar_tensor_tensor(
                            out=ps[:, Ls - P:Ls], in0=maskU[:], scalar=-1e30,
                            in1=ps[:, Ls - P:Ls], op0=MULT, op1=ADD) \
                            if kb == nb - 1 else None
                        mx = smp.tile([P, 1], F32, tag="mx", name="mx")
                        nc.vector.reduce_max(out=mx[:], in_=ps[:, :Ls], axis=AXX)
                        mxs.append(mx)
                        cp(S_sb[:, kb * 512:kb * 512 + Ls],
                                            ps[:, :Ls])
                    m = mxs[0]
                    for mx in mxs[1:]:
                        m2 = smp.tile([P, 1], F32, tag="mx", name="mx")
                        nc.vector.tensor_max(m2[:], m[:], mx[:])
                        m = m2
                    negm = smp.tile([P, 1], F32, tag="mx", name="mx")
                    nc.vector.tensor_scalar_mul(negm[:], m[:], -0.125)
                    denom = smp.tile([P, 1], F32, tag="mx", name="mx")
                    expS = Ep.tile([P, S], F32, tag="e", name="e")
                    nc.scalar.activation(expS[:, :L], S_sb[:, :L], EXP,
                                         bias=negm[:], scale=0.125,
                                         accum_out=denom[:])
                    recip = smp.tile([P, 1], F32, tag="mx", name="mx")
                    nc.vector.reciprocal(recip[:], denom[:])
                    att = psAV.tile([P, DH], F32, name="psAV")
                    nblk = L // P
                    for tb in range(nblk):
                        tp = psT.tile([P, P], F32, name="psT")
                        nc.tensor.transpose(tp[:], expS[:, tb * P:(tb + 1) * P],
                                            ident[:])
                        eT = Tp.tile([P, P], F32, tag="eT", name="eT")
                        cp(_r(eT[:]), tp[:])
                        nc.tensor.matmul(att[:], _r(eT[:]),
                                         _r(V_sb[tb][:, i * DH:(i + 1) * DH]),
                                         start=(tb == 0), stop=(tb == nblk - 1))
                    nc.vector.tensor_scalar_mul(
                        _r(pair[i // 2][:, (i % 2) * 64:(i % 2) * 64 + 64]),
                        att[:], recip[:])
                pairT = []
                for pr in range(2):
                    tp = psT.tile([P, P], F32, name="psT")
                    nc.tensor.transpose(tp[:], pair[pr][:], ident[:])
                    pT = Ap.tile([P, P], F32, tag="apT", name="apT")
                    cp(_r(pT[:]), tp[:])
                    pairT.append(pT)
                for d2h in range(2):
                    ps = psWO.tile([P, 512], F32, name="psWO")
                    for pr in range(2):
                        nc.tensor.matmul(
                            ps[:], _r(pairT[pr][:]),
                            _r(WOT_sb[pr][:, d2h * 512:(d2h + 1) * 512]),
                            start=(pr == 0), stop=(pr == 1))
                    osb = Op.tile([P, 512], F32, tag="osb", name="osb")
                    cp(osb[:], ps[:])
                    nc.sync.dma_start(
                        out=out_d[qt * P:(qt + 1) * P, d2h * 512:(d2h + 1) * 512],
                        in_=osb[:])
    nc.finalize()
    return nc


def kernel(x, fqk_weights_Q, fqk_weights_K, fv_weights,
           rqk_weights_Q, rqk_weights_K, rv_weights,
           f_qk, f_v, r_qk, r_v, W_O):
    x = np.ascontiguousarray(np.asarray(x, np.float32))
    F_qk = np.ascontiguousarray(
        np.asarray(f_qk, np.float32).transpose(1, 0, 2).reshape(D, NR))
    F_v = np.ascontiguousarray(
        np.asarray(f_v, np.float32).transpose(1, 0, 2).reshape(D, NR))
    R_qk = np.ascontiguousarray(np.asarray(r_qk, np.float32).reshape(NR, D))
    R_v = np.ascontiguousarray(np.asarray(r_v, np.float32).reshape(NR, D))
    W_OT = np.ascontiguousarray(np.asarray(W_O, np.float32).T)
    maskU = np.triu(np.full((P, P), 1.0, np.float32), 1)

    fw = [np.asarray(a, np.float32) for a in
          (fqk_weights_Q, fqk_weights_K, fv_weights)]
    rw = [np.asarray(a, np.float32) for a in
          (rqk_weights_Q, rqk_weights_K, rv_weights)]

    in_maps = []
    for c in range(8):
        b, ch = c // 4, c % 4
        t0 = ch * TOK
        hb = ch * HL  # first global head
        m = {
            "xT": np.ascontiguousarray(x[b, t0:t0 + TOK, :].T),
            "Fqk": F_qk, "Fv": F_v,
            "Rqk": np.ascontiguousarray(R_qk[:, hb * DH:hb * DH + DL]),
            "Rv": np.ascontiguousarray(R_v[:, hb * DH:hb * DH + DL]),
            "WOTs": np.ascontiguousarray(W_OT[hb * DH:hb * DH + DL, :]),
            "maskU": maskU,
        }
        for name, arr in zip(("fwq", "fwk", "fwv"), fw):
            m[name] = np.ascontiguousarray(arr[b, t0:t0 + TOK, :])
        for name, arr in zip(("rwqT", "rwkT", "rwvT"), rw):
            m[name] = np.ascontiguousarray(arr[b].T)
        in_maps.append(m)

    if "nc" not in _CACHED:
        _CACHED["nc"] = build()
    res = run_bass_kernel_spmd(_CACHED["nc"], in_maps, list(range(8)))
    out = np.zeros((B, S, D), np.float32)
    for c in range(8):
        out[c // 4] += res.results[c]["outp"]
    return out


if __name__ == "__main__":
    rng = np.random.RandomState(0)
    d = np.load("/tmp/inputs.npz")
    out = kernel(**{k: d[k] for k in d.files})
    ref = np.load("/tmp/ref_out.npy")
    rel = np.linalg.norm(out - ref) / np.linalg.norm(ref)
    print("rel fro err:", rel)
